# revision 34
# baseline (speedup 1.0000x reference)
"""Multi-head attention (B*H=64, S=2048, D=64) on 8 Trainium2 cores.

Sharding: 64 heads -> 8 per core (head-parallel, no communication).

Per-core kernel (heads processed in pairs A/B stacked on SBUF partition
halves 0:64 / 64:128):
  - prep (all pairs upfront): q/k are DMA'd with a 32x32-block-permuted
    access pattern, then a single DVE 32x32 block-transpose per tensor
    yields stacked Q^T/K^T [128(dA|dB), 2048] in natural q order; the
    fp16 rounding copy runs on GPSIMD (idle engine) for pairs > 0 to
    keep DVE free for exp.  V is loaded naturally, cast to bf16, and an
    appended ones column makes the PV matmul also produce the softmax
    denominator in row 64 (streaming P once covers both - the PE is
    rhs-stream-byte bound at ~800 GB/s, so a separate ones matmul that
    re-streams P costs more than the serial M=65 pair).
  - main loop per pair, flat over (q-chunk 512) x (16 k-tiles of 128):
      S^T[k,q] = K Q^T   -- two row-packed fp16 matmuls
                            (tile_position (0,0)/(64,0)), concurrent on PE
      P^T = exp(S^T)     -- bf16; exact exp on ACT for 9 k-tiles,
                            Schraudolph int16 bit-trick on DVE for 7,
                            strictly alternating engines except an ACT
                            run at kts 5-7 where DVE runs the previous
                            chunk's epilogue; exp turnaround (queue +
                            ~1.2us service) is the steady-state clock:
                            it gates PSUM s-bank recycling (PIPE=3 is
                            the PSUM cap) and the PV launches, so the
                            batch order leads with both QK prefetches
                            (exps launch at batch start and overlap the
                            PV streams) and next-pair prep on DVE is
                            priority-demoted below ready exps
      O^T[d+1,q] += V_aug^T P^T -- bf16 matmul, M=65, PSUM-accumulated;
                            row 64 accumulates the denominator Z
  - epilogue per (pair, q-chunk), interleaved into the next chunk's
    k-tile loop so no engine idles at the boundary: ACT-copies O^T to
    SBUF as bf16, PE-transposes back to [q, d+1] (bf16: half the f32
    transpose cost), DVE reciprocal of the Z column, one broadcast
    tensor_tensor multiply, DMA out.
"""

import os

import numpy as np

import concourse.bass as bass
import concourse.mybir as mybir
import concourse.tile as tile
from concourse import bacc
from concourse.bass_utils import run_bass_kernel_spmd
from concourse.masks import make_identity

B, S, D = 64, 2048, 64
NCORES = 8
H = B // NCORES  # heads per core
P = 128  # partitions
KT = S // P  # 16 k-tiles
QC = 512  # q-chunk
NQC = S // QC  # 4 q-chunks
NPAIR = H // 2  # head pairs per core
PIPE = 3  # k-tiles of QK^T in flight ahead of exp/PV

F32 = mybir.dt.float32
BF16 = mybir.dt.bfloat16
I16 = mybir.dt.int16
F16 = mybir.dt.float16

# k-tiles (of 16) whose exp runs on DVE via the Schraudolph bit trick
# (approximate, ~2% rms per weight); the rest run exact exp on ACT.
# One full [128, 2, 512] op per k-tile: splitting tiles into smaller ops
# was measured strictly worse (per-op fixed overhead ~350 ACT / ~130 DVE
# cycles plus sequencer dispatch dominates any latency win).
# kt 0 on DVE: at each q-chunk boundary ACT first drains the eviction
# copy of the previous chunk.  Keeping DVE at 7 k-tiles matters: 8 was
# measured 12us slower (DVE, which also carries the epilogue
# recip/mult and prep transposes, becomes the critical engine).  The
# engines strictly alternate except kts 5-7 (all ACT): that is where
# DVE runs the previous chunk's recip (kt 5) and normalize (kt 6), and
# it keeps the chunk TAIL alternating so exp(14)/exp(15) drain on both
# engines right before the boundary.
_DVE_KTS = {0, 2, 4, 8, 10, 12, 14}

# Schraudolph constants for bf16 exp via int16 bit pattern:
#   i = round_int16(x * 2^7/ln2 + b);  exp(x) ~= bitcast_bf16(i)
# b calibrated for round-to-nearest convert (max rel err ~3.3%).
_SCH_A = float(128.0 / np.log(2.0))
_SCH_B = float(os.environ.get("BASS_ATTN_SCH_B", "16250.5"))

# Engine for the prep rounding copies on pairs > 0: gpsimd or vector.
GPSIMD_CAST = os.environ.get("BASS_ATTN_GPSIMD_CAST", "1") == "1"


def build_attention_nc() -> bass.Bass:
    nc = bacc.Bacc()
    q_d = nc.declare_dram_parameter("q", [H, S, D], F32, isOutput=False)
    k_d = nc.declare_dram_parameter("k", [H, S, D], F32, isOutput=False)
    v_d = nc.declare_dram_parameter("v", [H, S, D], F32, isOutput=False)
    o_d = nc.declare_dram_parameter("out", [H, S, D], F32, isOutput=True)

    # 32x32-block-permuted views for the transpose loads:
    #   staging[32a + i, 32b + j] = x[32b + i, 32a + j]
    q_bp = q_d.rearrange("h (b i) (a j) -> h a i b j", i=32, j=32)
    k_bp = k_d.rearrange("h (b i) (a j) -> h a i b j", i=32, j=32)
    # natural views: row = t*128 + m (k index), row = g*512 + c*128 + p (q)
    v_v = v_d.rearrange("h (t p) d -> h p t d", p=P)
    o_v = o_d.rearrange("h (g c p) d -> h p g c d", c=4, p=P)

    with tile.TileContext(nc) as tc:
        with (
            tc.tile_pool(name="consts", bufs=1) as consts,
            tc.tile_pool(name="stage", bufs=4) as stage,
            tc.tile_pool(name="qk_t", bufs=16) as qkt_pool,
            tc.tile_pool(name="vpool", bufs=4) as vpool,
            tc.tile_pool(name="ppool", bufs=8) as ppool,
            tc.tile_pool(name="osb", bufs=4) as osb_pool,
            tc.tile_pool(name="outsb", bufs=4) as outsb_pool,
            tc.tile_pool(name="rz", bufs=4) as rz_pool,
            tc.tile_pool(name="spsum", bufs=PIPE, space="PSUM") as spsum,
            tc.tile_pool(name="oep", bufs=2, space="PSUM") as oep_pool,
        ):
            def emit_consts():
                ident65 = consts.tile([D + 1, D + 1], BF16)
                make_identity(nc, ident65[:])
                ones16 = consts.tile([P, KT], F32)
                nc.vector.memset(ones16[:], 1.0)
                # warm the ACT exp table-set while prep DMAs run
                warm = consts.tile([P, 4], F32)
                nc.gpsimd.memset(warm[:], 0.0)
                warm2 = consts.tile([P, 4], F32)
                nc.scalar.activation(
                    warm2[:], warm[:], mybir.ActivationFunctionType.Exp
                )
                return ident65, ones16

            def epilogue_copies(st):
                # PSUM -> SBUF eviction on ACT (bf16); frees the oep banks.
                # Both on ACT: DVE must stay clear for exp(kt0), which gates
                # both the first PVs and (via the s-bank ring) QK(kt3).
                for part in (0, 1):
                    o_sb = osb_pool.tile(
                        [D + 1, QC], BF16, tag="ot", name="o_sb"
                    )
                    nc.scalar.copy(o_sb[:], st["o_ps"][part][:])
                    st[f"o_sb{part}"] = o_sb

            def emit_exp(s_ps, p_sb, kt):
                if kt in _DVE_KTS:
                    nc.vector.tensor_scalar(
                        out=p_sb[:].bitcast(I16),
                        in0=s_ps[:],
                        scalar1=_SCH_A,
                        scalar2=_SCH_B,
                        op0=mybir.AluOpType.mult,
                        op1=mybir.AluOpType.add,
                    )
                else:
                    nc.scalar.activation(
                        p_sb[:], s_ps[:], mybir.ActivationFunctionType.Exp
                    )

            def epilogue_transposes(st):
                # transposed output for both heads shares one borrowed
                # s-pool slot: [128, 2, 4, 66] bf16 = 1056B <= the 4KB slot
                # (66, not 65: PSUM accesses must be 4-byte aligned)
                ep = spsum.tile([P, 2, 4, D + 2], BF16, tag="s", name="ep")
                st["ep"] = ep
                for part in (0, 1):
                    for c in range(4):
                        nc.tensor.transpose(
                            ep[:, part, c, 0 : D + 1],
                            st[f"o_sb{part}"][:, c * P : (c + 1) * P],
                            ident65[:],
                        )

            def epilogue_recip(st):
                rz = rz_pool.tile([P, 2, 4], F32, tag="rz", name="rz")
                st["rz"] = rz
                nc.vector.reciprocal(rz[:], st["ep"][:, :, :, D])

            def epilogue_norm(st):
                ep, rz, g = st["ep"], st["rz"], st["g"]
                out_sb = outsb_pool.tile(
                    [P, 2, 4, D], F32, tag="out", name="out_sb"
                )
                nc.vector.tensor_tensor(
                    out=out_sb[:],
                    in0=ep[:, :, :, 0:D],
                    in1=rz[:].broadcast_to([P, 2, 4, D]),
                    op=mybir.AluOpType.mult,
                )
                for part, hh in ((0, st["h_a"]), (1, st["h_b"])):
                    nc.sync.dma_start(
                        out=o_v[hh, :, g, :, :], in_=out_sb[:, part, :, :]
                    )

            def make_prep_units(pair, cast_eng, split_dma=False):
                """Per-pair prep as a list of emit-callbacks so the NEXT
                pair's prep can be interleaved into the CURRENT pair's
                k-tile loop (otherwise its transposes/casts queue behind
                all of this pair's DVE exp work and stall the seam).

                split_dma routes half of each chunk's staging descriptors
                through the ACT hardware DGE queue: only for pair 0, where
                descriptor generation (~0.7us per dma_start, serial on the
                sync sequencer) is on the kernel-start critical path and
                ACT is still idle."""
                h_a, h_b = 2 * pair, 2 * pair + 1
                qkt = {"q": [None] * 4, "k": [None] * 4}
                v_aug = {}
                units = []

                def v_unit(hh, part):
                    def emit():
                        vst = stage.tile([P, KT, D], F32, tag="vstage", name="vst")
                        nc.sync.dma_start(out=vst[:], in_=v_v[hh])
                        va = vpool.tile([P, KT, D + 1], BF16, tag="v", name="va")
                        cast_eng.tensor_copy(va[:, :, 0:D], vst[:])
                        cast_eng.tensor_copy(va[:, :, D], ones16[:])
                        v_aug[part] = va

                    return emit

                def chunk_unit(name, src, fc, ceng=None):
                    ceng = ceng or cast_eng
                    def emit():
                        st = stage.tile([P, QC], F32, tag="stage", name="st")
                        for hh, pb in ((h_a, 0), (h_b, 2)):
                            for a in range(2):
                                c = pb + a
                                dma_eng = nc.scalar if (split_dma and a == 1) else nc.sync
                                dma_eng.dma_start(
                                    out=st[32 * c : 32 * c + 32, :].rearrange(
                                        "i (b j) -> i b j", j=32
                                    ),
                                    in_=src[hh, a, :, 16 * fc : 16 * fc + 16, :],
                                )
                        st2 = stage.tile([P, QC], F32, tag="stage2", name="st2")
                        nc.vector.transpose(st2[:], st[:])
                        ch = qkt_pool.tile([P, QC], F16, tag="qkT", name="ch")
                        ceng.tensor_copy(ch[:], st2[:])
                        qkt[name][fc] = ch

                    return emit

                # k0/q0 first (gate the first QK), then V (needed by the
                # first PV, shortly after), then the remaining chunks.
                units.append(chunk_unit("k", k_bp, 0))
                units.append(chunk_unit("q", q_bp, 0))
                units.append(v_unit(h_a, 0))
                units.append(v_unit(h_b, 1))
                for fc in range(1, 4):
                    units.append(chunk_unit("k", k_bp, fc))
                    units.append(chunk_unit("q", q_bp, fc))
                return units, qkt, v_aug

            # pair 0's prep is on the critical path at kernel start: emit it
            # upfront with the faster DVE doing the rounding casts, k0/q0
            # staging DMAs first so descriptor generation starts at time
            # zero (consts touch no DGE queue and slot in behind).
            units, qkt, v_aug = make_prep_units(0, nc.vector, split_dma=True)
            units[0]()
            units[1]()
            ident65, ones16 = emit_consts()
            for u in units[2:]:
                u()

            pend = None  # epilogue state of the previous q-chunk
            for pair in range(NPAIR):
                kT = qkt["k"]
                qT = qkt["q"]
                nxt_units = []
                if pair + 1 < NPAIR:
                    nxt_units, nxt_qkt, nxt_v = make_prep_units(
                        pair + 1, nc.gpsimd if GPSIMD_CAST else nc.vector
                    )
                ui = 0

                # ---------------- main ----------------
                # Flat software pipeline over (g, kt): QK^T runs PIPE k-tiles
                # ahead (crossing q-chunk boundaries), the previous chunk's
                # epilogue is interleaved into the first k-tiles of the next
                # chunk, and the next pair's prep units are spread across
                # this pair's loop.
                TT = NQC * KT

                def emit_qkt(t, kT=kT, qT=qT):
                    gq, kt = divmod(t, KT)
                    s_ps = spsum.tile([P, 2, QC], F32, tag="s", name="s_ps")
                    k_ch = kT[kt // 4]
                    k_sl = slice((kt % 4) * P, (kt % 4 + 1) * P)
                    for part, base in ((0, 0), (1, 64)):
                        nc.tensor.matmul(
                            s_ps[:, part, :],
                            k_ch[base : base + 64, k_sl],
                            qT[gq][base : base + 64, :],
                            tile_position=(base, 0),
                        )
                    return s_ps

                s_tiles = {t: emit_qkt(t) for t in range(PIPE)}
                o_ps = None
                defer_pv = []
                for t in range(TT):
                    g, kt = divmod(t, KT)
                    if kt == 0:
                        if pend is not None:
                            epilogue_copies(pend)
                        o_ps = [
                            oep_pool.tile([D + 1, QC], F32, tag="oep", name="o_ps"),
                            oep_pool.tile([D + 1, QC], F32, tag="oep", name="o_ps"),
                        ]
                    s_ps = s_tiles.pop(t)
                    p_sb = ppool.tile([P, 2, QC], BF16, tag="p")
                    emit_exp(s_ps, p_sb, kt)
                    # 2-kt PE batching with split-ends QK placement:
                    # [QK(t+2) | PV x4 | QK(t+3)] -- same 2 group switches
                    # per batch as all-PVs-then-all-QKs, but QK(t+2) leads
                    # the batch: its s-bank was freed a batch ago, so it
                    # never stalls, and the next k-tile's exp launches
                    # ~0.9us earlier, overlapping this batch's PV streams
                    # instead of serializing after them.  QK(t+3) trails
                    # the PVs (its WAR on s_ps(t) clears when exp(t) ends,
                    # which the PVs of kt=t just consumed).
                    defer_pv.append((o_ps, p_sb, kt))
                    if kt % 2 == 1:
                        for td in (t - 1 + PIPE, t + PIPE):
                            if td < TT:
                                s_tiles[td] = emit_qkt(td)
                        for ops_d, p_d, kt_d in defer_pv:
                            for part in (0, 1):
                                nc.tensor.matmul(
                                    ops_d[part][:],
                                    v_aug[part][:, kt_d, :],
                                    p_d[:, part, :],
                                    start=(kt_d == 0),
                                    stop=(kt_d == KT - 1),
                                )
                        defer_pv = []
                    if ui < len(nxt_units) and t == 2 + 6 * ui:
                        # Demote prep priority: when the DVE (or a DGE
                        # queue) is free it should always prefer a ready
                        # exp over next-pair prep, which has a whole
                        # pair-duration of slack.
                        with tc.high_priority(offset=-1_000_000):
                            nxt_units[ui]()
                        ui += 1
                    if pend is not None:
                        if kt == 4:
                            epilogue_transposes(pend)
                        elif kt == 5:
                            epilogue_recip(pend)
                        elif kt == 6:
                            epilogue_norm(pend)
                            pend = None
                    if kt == KT - 1:
                        pend = {
                            "o_ps": o_ps,
                            "g": g,
                            "h_a": 2 * pair,
                            "h_b": 2 * pair + 1,
                        }
                while ui < len(nxt_units):
                    nxt_units[ui]()
                    ui += 1
                if pair + 1 < NPAIR:
                    qkt, v_aug = nxt_qkt, nxt_v
            # tail: epilogue of the very last q-chunk.  No exp work remains,
            # so split the evictions across ACT and DVE to shorten the
            # serial tail chain.
            for part in (0, 1):
                o_sb = osb_pool.tile([D + 1, QC], BF16, tag="ot", name="o_sb")
                if part == 0:
                    nc.scalar.copy(o_sb[:], pend["o_ps"][part][:])
                else:
                    nc.vector.tensor_copy(o_sb[:], pend["o_ps"][part][:])
                pend[f"o_sb{part}"] = o_sb
            epilogue_transposes(pend)
            epilogue_recip(pend)
            epilogue_norm(pend)
            pend = None
    nc.finalize()
    return nc


_NC_CACHE = None


def _get_nc():
    global _NC_CACHE
    if _NC_CACHE is None:
        _NC_CACHE = build_attention_nc()
    return _NC_CACHE


def kernel(q: np.ndarray, k: np.ndarray, v: np.ndarray) -> np.ndarray:
    q = np.asarray(q, dtype=np.float32)
    k = np.asarray(k, dtype=np.float32)
    v = np.asarray(v, dtype=np.float32)
    nc = _get_nc()
    in_maps = [
        {
            "q": np.ascontiguousarray(q[c * H : (c + 1) * H]),
            "k": np.ascontiguousarray(k[c * H : (c + 1) * H]),
            "v": np.ascontiguousarray(v[c * H : (c + 1) * H]),
        }
        for c in range(NCORES)
    ]
    res = run_bass_kernel_spmd(nc, in_maps, list(range(NCORES)))
    return np.concatenate([res.results[c]["out"] for c in range(NCORES)], axis=0)



# revision 35
# speedup vs baseline: 1.0080x; 1.0080x over previous
"""Multi-head attention (B*H=64, S=2048, D=64) on 8 Trainium2 cores.

Sharding: 64 heads -> 8 per core (head-parallel, no communication).

Per-core kernel (heads processed in pairs A/B stacked on SBUF partition
halves 0:64 / 64:128):
  - prep (all pairs upfront): q/k are DMA'd with a 32x32-block-permuted
    access pattern, then a single DVE 32x32 block-transpose per tensor
    yields stacked Q^T/K^T [128(dA|dB), 2048] in natural q order; the
    fp16 rounding copy runs on GPSIMD (idle engine) for pairs > 0 to
    keep DVE free for exp.  V is loaded naturally, cast to bf16, and an
    appended ones column makes the PV matmul also produce the softmax
    denominator in row 64 (streaming P once covers both - the PE is
    rhs-stream-byte bound at ~800 GB/s, so a separate ones matmul that
    re-streams P costs more than the serial M=65 pair).
  - main loop per pair, flat over (q-chunk 512) x (16 k-tiles of 128):
      S^T[k,q] = K Q^T   -- two row-packed fp16 matmuls
                            (tile_position (0,0)/(64,0)), concurrent on PE
      P^T = exp(S^T)     -- bf16; exact exp on ACT for 9 k-tiles,
                            Schraudolph int16 bit-trick on DVE for 7,
                            strictly alternating engines except an ACT
                            run at kts 5-7 where DVE runs the previous
                            chunk's epilogue; exp turnaround (queue +
                            ~1.2us service) is the steady-state clock:
                            it gates PSUM s-bank recycling (PIPE=3 is
                            the PSUM cap) and the PV launches, so the
                            batch order leads with both QK prefetches
                            (exps launch at batch start and overlap the
                            PV streams) and next-pair prep on DVE is
                            priority-demoted below ready exps
      O^T[d+1,q] += V_aug^T P^T -- bf16 matmul, M=65, PSUM-accumulated;
                            row 64 accumulates the denominator Z
  - epilogue per (pair, q-chunk), interleaved into the next chunk's
    k-tile loop so no engine idles at the boundary: ACT-copies O^T to
    SBUF as bf16, PE-transposes back to [q, d+1] (bf16: half the f32
    transpose cost), DVE reciprocal of the Z column, one broadcast
    tensor_tensor multiply, DMA out.
"""

import os

import numpy as np

import concourse.bass as bass
import concourse.mybir as mybir
import concourse.tile as tile
from concourse import bacc
from concourse.bass_utils import run_bass_kernel_spmd
from concourse.masks import make_identity

B, S, D = 64, 2048, 64
NCORES = 8
H = B // NCORES  # heads per core
P = 128  # partitions
KT = S // P  # 16 k-tiles
QC = 512  # q-chunk
NQC = S // QC  # 4 q-chunks
NPAIR = H // 2  # head pairs per core
PIPE = 3  # k-tiles of QK^T in flight ahead of exp/PV

F32 = mybir.dt.float32
BF16 = mybir.dt.bfloat16
I16 = mybir.dt.int16
F16 = mybir.dt.float16

# k-tiles (of 16) whose exp runs on DVE via the Schraudolph bit trick
# (approximate, ~2% rms per weight); the rest run exact exp on ACT.
# One full [128, 2, 512] op per k-tile: splitting tiles into smaller ops
# was measured strictly worse (per-op fixed overhead ~350 ACT / ~130 DVE
# cycles plus sequencer dispatch dominates any latency win).
# kt 0 on DVE: at each q-chunk boundary ACT first drains the eviction
# copy of the previous chunk.  Keeping DVE at 7 k-tiles matters: 8 was
# measured 12us slower (DVE, which also carries the epilogue
# recip/mult and prep transposes, becomes the critical engine).  The
# engines strictly alternate except kts 5-7 (all ACT): that is where
# DVE runs the previous chunk's recip (kt 5) and normalize (kt 6), and
# it keeps the chunk TAIL alternating so exp(14)/exp(15) drain on both
# engines right before the boundary.
_DVE_KTS = {0, 2, 4, 8, 10, 12, 14}

# Schraudolph constants for bf16 exp via int16 bit pattern:
#   i = round_int16(x * 2^7/ln2 + b);  exp(x) ~= bitcast_bf16(i)
# b calibrated for round-to-nearest convert (max rel err ~3.3%).
_SCH_A = float(128.0 / np.log(2.0))
_SCH_B = float(os.environ.get("BASS_ATTN_SCH_B", "16250.5"))

# Engine for the prep rounding copies on pairs > 0: gpsimd or vector.
GPSIMD_CAST = os.environ.get("BASS_ATTN_GPSIMD_CAST", "1") == "1"


def build_attention_nc() -> bass.Bass:
    nc = bacc.Bacc()
    q_d = nc.declare_dram_parameter("q", [H, S, D], F32, isOutput=False)
    k_d = nc.declare_dram_parameter("k", [H, S, D], F32, isOutput=False)
    v_d = nc.declare_dram_parameter("v", [H, S, D], F32, isOutput=False)
    o_d = nc.declare_dram_parameter("out", [H, S, D], F32, isOutput=True)

    # 32x32-block-permuted views for the transpose loads:
    #   staging[32a + i, 32b + j] = x[32b + i, 32a + j]
    q_bp = q_d.rearrange("h (b i) (a j) -> h a i b j", i=32, j=32)
    k_bp = k_d.rearrange("h (b i) (a j) -> h a i b j", i=32, j=32)
    # natural views: row = t*128 + m (k index), row = g*512 + c*128 + p (q)
    v_v = v_d.rearrange("h (t p) d -> h p t d", p=P)
    o_v = o_d.rearrange("h (g c p) d -> h p g c d", c=4, p=P)

    with tile.TileContext(nc) as tc:
        with (
            tc.tile_pool(name="consts", bufs=1) as consts,
            tc.tile_pool(name="stage", bufs=4) as stage,
            tc.tile_pool(name="qk_t", bufs=16) as qkt_pool,
            tc.tile_pool(name="vpool", bufs=4) as vpool,
            tc.tile_pool(name="ppool", bufs=8) as ppool,
            tc.tile_pool(name="osb", bufs=4) as osb_pool,
            tc.tile_pool(name="outsb", bufs=4) as outsb_pool,
            tc.tile_pool(name="rz", bufs=4) as rz_pool,
            tc.tile_pool(name="spsum", bufs=PIPE, space="PSUM") as spsum,
            tc.tile_pool(name="oep", bufs=2, space="PSUM") as oep_pool,
        ):
            def emit_consts():
                ident65 = consts.tile([D + 1, D + 1], BF16)
                make_identity(nc, ident65[:])
                ones16 = consts.tile([P, KT], F32)
                nc.vector.memset(ones16[:], 1.0)
                # warm the ACT exp table-set while prep DMAs run
                warm = consts.tile([P, 4], F32)
                nc.gpsimd.memset(warm[:], 0.0)
                warm2 = consts.tile([P, 4], F32)
                nc.scalar.activation(
                    warm2[:], warm[:], mybir.ActivationFunctionType.Exp
                )
                return ident65, ones16

            def epilogue_copies(st):
                # PSUM -> SBUF eviction on ACT (bf16); frees the oep banks.
                # Both on ACT: DVE must stay clear for exp(kt0), which gates
                # both the first PVs and (via the s-bank ring) QK(kt3).
                for part in (0, 1):
                    o_sb = osb_pool.tile(
                        [D + 1, QC], BF16, tag="ot", name="o_sb"
                    )
                    nc.scalar.copy(o_sb[:], st["o_ps"][part][:])
                    st[f"o_sb{part}"] = o_sb

            def emit_exp(s_ps, p_sb, kt):
                if kt in _DVE_KTS:
                    nc.vector.tensor_scalar(
                        out=p_sb[:].bitcast(I16),
                        in0=s_ps[:],
                        scalar1=_SCH_A,
                        scalar2=_SCH_B,
                        op0=mybir.AluOpType.mult,
                        op1=mybir.AluOpType.add,
                    )
                else:
                    nc.scalar.activation(
                        p_sb[:], s_ps[:], mybir.ActivationFunctionType.Exp
                    )

            def epilogue_transposes(st):
                # transposed output for both heads shares one borrowed
                # s-pool slot: [128, 2, 4, 66] bf16 = 1056B <= the 4KB slot
                # (66, not 65: PSUM accesses must be 4-byte aligned)
                ep = spsum.tile([P, 2, 4, D + 2], BF16, tag="s", name="ep")
                st["ep"] = ep
                for part in (0, 1):
                    for c in range(4):
                        nc.tensor.transpose(
                            ep[:, part, c, 0 : D + 1],
                            st[f"o_sb{part}"][:, c * P : (c + 1) * P],
                            ident65[:],
                        )

            def epilogue_recip(st):
                rz = rz_pool.tile([P, 2, 4], F32, tag="rz", name="rz")
                st["rz"] = rz
                nc.vector.reciprocal(rz[:], st["ep"][:, :, :, D])

            def epilogue_norm(st):
                ep, rz, g = st["ep"], st["rz"], st["g"]
                out_sb = outsb_pool.tile(
                    [P, 2, 4, D], F32, tag="out", name="out_sb"
                )
                nc.vector.tensor_tensor(
                    out=out_sb[:],
                    in0=ep[:, :, :, 0:D],
                    in1=rz[:].broadcast_to([P, 2, 4, D]),
                    op=mybir.AluOpType.mult,
                )
                for part, hh in ((0, st["h_a"]), (1, st["h_b"])):
                    nc.sync.dma_start(
                        out=o_v[hh, :, g, :, :], in_=out_sb[:, part, :, :]
                    )

            def make_prep_units(pair, cast_eng, split_dma=False):
                """Per-pair prep as a list of emit-callbacks so the NEXT
                pair's prep can be interleaved into the CURRENT pair's
                k-tile loop (otherwise its transposes/casts queue behind
                all of this pair's DVE exp work and stall the seam).

                split_dma routes half of each chunk's staging descriptors
                through the ACT hardware DGE queue: only for pair 0, where
                descriptor generation (~0.7us per dma_start, serial on the
                sync sequencer) is on the kernel-start critical path and
                ACT is still idle."""
                h_a, h_b = 2 * pair, 2 * pair + 1
                qkt = {"q": [None] * 4, "k": [None] * 4}
                v_aug = {}
                units = []

                def v_unit(hh, part):
                    def emit():
                        vst = stage.tile([P, KT, D], F32, tag="vstage", name="vst")
                        nc.sync.dma_start(out=vst[:], in_=v_v[hh])
                        va = vpool.tile([P, KT, D + 1], BF16, tag="v", name="va")
                        cast_eng.tensor_copy(va[:, :, 0:D], vst[:])
                        cast_eng.tensor_copy(va[:, :, D], ones16[:])
                        v_aug[part] = va

                    return emit

                def chunk_unit(name, src, fc, ceng=None):
                    ceng = ceng or cast_eng
                    def emit():
                        st = stage.tile([P, QC], F32, tag="stage", name="st")
                        for hh, pb in ((h_a, 0), (h_b, 2)):
                            for a in range(2):
                                c = pb + a
                                dma_eng = nc.scalar if (split_dma and a == 1) else nc.sync
                                dma_eng.dma_start(
                                    out=st[32 * c : 32 * c + 32, :].rearrange(
                                        "i (b j) -> i b j", j=32
                                    ),
                                    in_=src[hh, a, :, 16 * fc : 16 * fc + 16, :],
                                )
                        # cast BEFORE the block-transpose: the DVE stream
                        # transpose moves half the bytes in fp16
                        st2 = stage.tile([P, QC], F16, tag="stage2", name="st2")
                        ceng.tensor_copy(st2[:], st[:])
                        ch = qkt_pool.tile([P, QC], F16, tag="qkT", name="ch")
                        nc.vector.transpose(ch[:], st2[:])
                        qkt[name][fc] = ch

                    return emit

                # k0/q0 first (gate the first QK), then V (needed by the
                # first PV, shortly after), then the remaining chunks.
                units.append(chunk_unit("k", k_bp, 0))
                units.append(chunk_unit("q", q_bp, 0))
                units.append(v_unit(h_a, 0))
                units.append(v_unit(h_b, 1))
                for fc in range(1, 4):
                    units.append(chunk_unit("k", k_bp, fc))
                    units.append(chunk_unit("q", q_bp, fc))
                return units, qkt, v_aug

            # pair 0's prep is on the critical path at kernel start: emit it
            # upfront with the faster DVE doing the rounding casts, k0/q0
            # staging DMAs first so descriptor generation starts at time
            # zero (consts touch no DGE queue and slot in behind).
            units, qkt, v_aug = make_prep_units(0, nc.vector, split_dma=True)
            units[0]()
            units[1]()
            ident65, ones16 = emit_consts()
            for u in units[2:]:
                u()

            pend = None  # epilogue state of the previous q-chunk
            for pair in range(NPAIR):
                kT = qkt["k"]
                qT = qkt["q"]
                nxt_units = []
                if pair + 1 < NPAIR:
                    nxt_units, nxt_qkt, nxt_v = make_prep_units(
                        pair + 1, nc.gpsimd if GPSIMD_CAST else nc.vector
                    )
                ui = 0

                # ---------------- main ----------------
                # Flat software pipeline over (g, kt): QK^T runs PIPE k-tiles
                # ahead (crossing q-chunk boundaries), the previous chunk's
                # epilogue is interleaved into the first k-tiles of the next
                # chunk, and the next pair's prep units are spread across
                # this pair's loop.
                TT = NQC * KT

                def emit_qkt(t, kT=kT, qT=qT):
                    gq, kt = divmod(t, KT)
                    s_ps = spsum.tile([P, 2, QC], F32, tag="s", name="s_ps")
                    k_ch = kT[kt // 4]
                    k_sl = slice((kt % 4) * P, (kt % 4 + 1) * P)
                    for part, base in ((0, 0), (1, 64)):
                        nc.tensor.matmul(
                            s_ps[:, part, :],
                            k_ch[base : base + 64, k_sl],
                            qT[gq][base : base + 64, :],
                            tile_position=(base, 0),
                        )
                    return s_ps

                s_tiles = {t: emit_qkt(t) for t in range(PIPE)}
                o_ps = None
                defer_pv = []
                for t in range(TT):
                    g, kt = divmod(t, KT)
                    if kt == 0:
                        if pend is not None:
                            epilogue_copies(pend)
                        o_ps = [
                            oep_pool.tile([D + 1, QC], F32, tag="oep", name="o_ps"),
                            oep_pool.tile([D + 1, QC], F32, tag="oep", name="o_ps"),
                        ]
                    s_ps = s_tiles.pop(t)
                    p_sb = ppool.tile([P, 2, QC], BF16, tag="p")
                    emit_exp(s_ps, p_sb, kt)
                    # 2-kt PE batching with split-ends QK placement:
                    # [QK(t+2) | PV x4 | QK(t+3)] -- same 2 group switches
                    # per batch as all-PVs-then-all-QKs, but QK(t+2) leads
                    # the batch: its s-bank was freed a batch ago, so it
                    # never stalls, and the next k-tile's exp launches
                    # ~0.9us earlier, overlapping this batch's PV streams
                    # instead of serializing after them.  QK(t+3) trails
                    # the PVs (its WAR on s_ps(t) clears when exp(t) ends,
                    # which the PVs of kt=t just consumed).
                    defer_pv.append((o_ps, p_sb, kt))
                    if kt % 2 == 1:
                        for td in (t - 1 + PIPE, t + PIPE):
                            if td < TT:
                                s_tiles[td] = emit_qkt(td)
                        for ops_d, p_d, kt_d in defer_pv:
                            for part in (0, 1):
                                nc.tensor.matmul(
                                    ops_d[part][:],
                                    v_aug[part][:, kt_d, :],
                                    p_d[:, part, :],
                                    start=(kt_d == 0),
                                    stop=(kt_d == KT - 1),
                                )
                        defer_pv = []
                    if ui < len(nxt_units) and t == 2 + 6 * ui:
                        # Demote prep priority: when the DVE (or a DGE
                        # queue) is free it should always prefer a ready
                        # exp over next-pair prep, which has a whole
                        # pair-duration of slack.
                        with tc.high_priority(offset=-1_000_000):
                            nxt_units[ui]()
                        ui += 1
                    if pend is not None:
                        if kt == 4:
                            epilogue_transposes(pend)
                        elif kt == 5:
                            epilogue_recip(pend)
                        elif kt == 6:
                            epilogue_norm(pend)
                            pend = None
                    if kt == KT - 1:
                        pend = {
                            "o_ps": o_ps,
                            "g": g,
                            "h_a": 2 * pair,
                            "h_b": 2 * pair + 1,
                        }
                while ui < len(nxt_units):
                    nxt_units[ui]()
                    ui += 1
                if pair + 1 < NPAIR:
                    qkt, v_aug = nxt_qkt, nxt_v
            # tail: epilogue of the very last q-chunk.  No exp work remains,
            # so split the evictions across ACT and DVE to shorten the
            # serial tail chain.
            for part in (0, 1):
                o_sb = osb_pool.tile([D + 1, QC], BF16, tag="ot", name="o_sb")
                if part == 0:
                    nc.scalar.copy(o_sb[:], pend["o_ps"][part][:])
                else:
                    nc.vector.tensor_copy(o_sb[:], pend["o_ps"][part][:])
                pend[f"o_sb{part}"] = o_sb
            epilogue_transposes(pend)
            epilogue_recip(pend)
            epilogue_norm(pend)
            pend = None
    nc.finalize()
    return nc


_NC_CACHE = None


def _get_nc():
    global _NC_CACHE
    if _NC_CACHE is None:
        _NC_CACHE = build_attention_nc()
    return _NC_CACHE


def kernel(q: np.ndarray, k: np.ndarray, v: np.ndarray) -> np.ndarray:
    q = np.asarray(q, dtype=np.float32)
    k = np.asarray(k, dtype=np.float32)
    v = np.asarray(v, dtype=np.float32)
    nc = _get_nc()
    in_maps = [
        {
            "q": np.ascontiguousarray(q[c * H : (c + 1) * H]),
            "k": np.ascontiguousarray(k[c * H : (c + 1) * H]),
            "v": np.ascontiguousarray(v[c * H : (c + 1) * H]),
        }
        for c in range(NCORES)
    ]
    res = run_bass_kernel_spmd(nc, in_maps, list(range(NCORES)))
    return np.concatenate([res.results[c]["out"] for c in range(NCORES)], axis=0)

